# revision 55
# baseline (speedup 1.0000x reference)
"""Log2Quantizer Trainium2 kernel (raw Bass, no Tile).

Math: the reference's sort/std/rank machinery is dead code (bit_token is
unconditionally overwritten with n_bits), so the computation reduces to:
    delta[b,t] = max over (h,c) of x[b,h,t,c]
    out = delta * 2^(round(log2(max(x/delta, 1e-8))))
i.e. snap x/delta to the nearest power of two in log space, rescale by delta.

Bit-trick (no transcendentals): round(log2 r) = floor(log2(r/sqrt2)) + 1:
    q   = x * (isqrt2/delta)                 per-token scale (ACT, M1)
    p2  = bitcast_f32(bits(q) & 0x7F800000)  2^floor(log2 q)   (DVE, AND)
    out = p2 * (2*delta)                     exact fp32 mult   (M2, split)
x==0 gives q=0 -> p2=+0.0 -> out=0 (reference's clamp yields delta*2^-27
~ 7e-9 there; abs err 7e-9 on the rare exact-zero input).

Sharding: data-parallel over batch dim b (8 rows -> 8 cores), no comms.
Layout: t split into chunks; partition dim = t-block of tt=TC/128 so each
partition line is one contiguous tt*256B run per h in DRAM (>=512B keeps
DMA descriptors at the full 22.5 B/ns per-engine bus rate; 16 engines ~
360 GB/s aggregate is the hard bottleneck at 25.2 MB total traffic ->
~70us floor). Chunk sizes are VARIABLE: small 256-token chunks at the
start (first store enters the DMA mix ~20us earlier, so the load/store
streams share the engines for more of the run) and at the end (the tail
drains a small last store instead of a 1.6MB one).

Engine pipeline (vs the all-DVE baseline at 98.6us whose DVE ran 86us).
Per-token-scalar ops are sliced into tt per-q ops whose scalar is a
[128,1] AP; measured slice costs: DVE ~613ns, ACT ~1017ns per 512-chunk
slice.
  Sync:   load DMAs (own HWDGE ring), paced to <=3 in flight: an overfull
          ring makes dma_start block and throttles the transfers (measured
          260 GB/s un-paced vs 350 paced). One SBUF buffer pair per chunk
          (whole tensor resident), so loads wait on nothing else.
  DVE:    2-stage max-reduce (contiguous-X over c at 2 elem/cyc, then the
          tiny strided reduce over h), reciprocal, 2 tiny per-token ops,
          flat single-op AND, M2 slices 0..m2d-1.
  ACT:    M1 (all tt slices) + M2 slices m2d..tt-1 (activation-Copy,
          per-partition scale AP) + store DMA issue (own HWDGE ring).
Per 512-chunk: DVE ~7.5us, ACT ~7.1us, within the ~8.1us/chunk DMA share,
so in steady state the DMA rings are the pacer, not compute.

Buffers ping-pong, no in-place ops, one pair per chunk so no reuse:
M1 xt->wt, AND wt->xt, M2 xt->wt, store from wt.

Sems -- every data handoff (same-engine included: engines pipeline, e.g.
an issued DMA can read an earlier op's output before its data lands)
waits on the producer's counting increment (cumulative over chunks since
slice counts vary):
  load_sem:  +16 per load DMA; DVE waits 16*(ci+1) (one FIFO ring, in-order)
  scal_sem:  +2 per chunk by DVE (inv', d2); ACT M1 waits 2*(ci+1)
  m1_sem:    +1 per ACT M1 slice; DVE AND waits cum_m1(ci)
  and_sem:   +1 per chunk by DVE AND; ACT M2b waits ci+1
  m2a_sem:   +1 per DVE M2 slice;  ACT store waits cum_m2a(ci)
  m2b_sem:   +1 per ACT M2 slice;  ACT store self-waits cum_m2b(ci)
  dve_sem:   DVE-internal RAW fences (reduce1->reduce2->recip->tinies)
  store_sem: +16 per store DMA; sync tail-waits 16*n_chunks (output flushed)
"""

from contextlib import ExitStack

import numpy as np

import concourse.bass as bass
import concourse.mybir as mybir
from concourse.bass_utils import run_bass_kernel_spmd

B, H, T, C = 8, 12, 4096, 64
N_CORES = 8
P = 128          # SBUF partitions

# chunk sizes in tokens; sum must be T. Small chunks at both ends: the
# first store enters the DMA mix early, and the tail drains a small store.
# minimum 256 tokens: tt=2 keeps per-(partition,h) DRAM runs at 512B, the
# smallest size that still runs the DMA descriptors at full bus rate
CHUNKS = [256, 256, 512, 512, 512, 512, 512, 512, 256, 256]
assert sum(CHUNKS) == T

ISQRT2 = 0.7071067811865476
EXP_MASK = 0x7F800000

_nc_cache = {}


def _build_nc():
    if "nc" in _nc_cache:
        return _nc_cache["nc"]
    f32 = mybir.dt.float32
    i32 = mybir.dt.int32
    OP = mybir.AluOpType
    AF = mybir.ActivationFunctionType

    nc = bass.Bass()
    x_in = nc.declare_dram_parameter("x", [H, T, C], f32, isOutput=False)
    y_out = nc.declare_dram_parameter("y", [H, T, C], f32, isOutput=True)

    n_chunks = len(CHUNKS)
    offs = [sum(CHUNKS[:i]) for i in range(n_chunks)]
    tts = [tc // P for tc in CHUNKS]
    m2d = [tt // 2 for tt in tts]          # M2 slices on DVE per chunk

    def cum(xs, i):
        # total of xs[0..i] inclusive
        return sum(xs[: i + 1])

    def src_ap(ci):
        return x_in[:, offs[ci] : offs[ci] + CHUNKS[ci], :].rearrange(
            "h (p q) c -> p h (q c)", p=P
        )

    def dst_ap(ci):
        return y_out[:, offs[ci] : offs[ci] + CHUNKS[ci], :].rearrange(
            "h (p q) c -> p h (q c)", p=P
        )

    with ExitStack() as ctx:
        xt = [
            ctx.enter_context(
                nc.sbuf_tensor(f"xt{j}", [P, H * tts[j] * C], f32)
            )
            for j in range(n_chunks)
        ]
        wt = [
            ctx.enter_context(
                nc.sbuf_tensor(f"wt{j}", [P, H * tts[j] * C], f32)
            )
            for j in range(n_chunks)
        ]
        # scratch for the c-reduce stage
        red2 = ctx.enter_context(
            nc.sbuf_tensor("red2", [P, H * max(tts)], f32)
        )
        delta = [
            ctx.enter_context(nc.sbuf_tensor(f"delta{j}", [P, tts[j]], f32))
            for j in range(n_chunks)
        ]
        inv = [
            ctx.enter_context(nc.sbuf_tensor(f"inv{j}", [P, tts[j]], f32))
            for j in range(n_chunks)
        ]
        d2 = [
            ctx.enter_context(nc.sbuf_tensor(f"d2_{j}", [P, tts[j]], f32))
            for j in range(n_chunks)
        ]

        load_sem = ctx.enter_context(nc.semaphore("load_sem"))
        store_sem = ctx.enter_context(nc.semaphore("store_sem"))
        scal_sem = ctx.enter_context(nc.semaphore("scal_sem"))
        m1_sem = ctx.enter_context(nc.semaphore("m1_sem"))
        and_sem = ctx.enter_context(nc.semaphore("and_sem"))
        m2a_sem = ctx.enter_context(nc.semaphore("m2a_sem"))
        m2b_sem = ctx.enter_context(nc.semaphore("m2b_sem"))
        dve_sem = ctx.enter_context(nc.semaphore("dve_sem"))

        block = ctx.enter_context(nc.Block())

        def view4(t, ci):
            return t[:].rearrange("p (h q c) -> p h q c", h=H, c=C)

        cm2b = [tts[i] - m2d[i] for i in range(n_chunks)]

        @block.sync
        def _(sync):
            # loads only; SP HWDGE ring (two-ring DMA splits tested net-
            # negative: the ~350 GB/s single-stream cap is an HBM R/W-
            # direction effect, not per-ring -- mixed R+W reaches ~390)
            for ci in range(n_chunks):
                if ci >= 3:
                    sync.wait_ge(load_sem, 16 * (ci - 2))
                sync.dma_start(out=xt[ci][:], in_=src_ap(ci)).then_inc(
                    load_sem, 16
                )
            # output-flush guarantee before NEFF end
            sync.wait_ge(store_sem, 16 * n_chunks)

        @block.vector
        def _(vector):
            def and_m2a(k):
                # AND: p2 = bits(q) & mask, wt -> xt, one flat op
                vector.wait_ge(m1_sem, cum(tts, k))
                vector.tensor_scalar(
                    out=xt[k][:].bitcast(i32),
                    in0=wt[k][:].bitcast(i32),
                    scalar1=EXP_MASK,
                    scalar2=None,
                    op0=OP.bitwise_and,
                ).then_inc(and_sem, 1)
                # M2 slices 0..m2d-1: out = p2 * d2, xt -> wt
                vector.wait_ge(and_sem, k + 1)
                for s in range(m2d[k]):
                    vector.tensor_scalar_mul(
                        view4(wt[k], k)[:, :, s, :],
                        view4(xt[k], k)[:, :, s, :],
                        d2[k][:, s : s + 1],
                    ).then_inc(m2a_sem, 1)

            b = 0
            for ci in range(n_chunks):
                tt = tts[ci]
                vector.wait_ge(load_sem, 16 * (ci + 1))
                # delta = max over (h, c) in two stages: contiguous X over
                # c, then the tiny strided reduce over h. (Alternatives
                # measured SLOWER: transposed XY reduce 3346ns/chunk,
                # tensor_scalar+accum 'CACHE_REDUCE' 948ns/slice, h-fold
                # via tensor_max 1432ns + half-reduce; this is 3584ns.)
                vector.reduce_max(
                    out=red2[:, : H * tt],
                    in_=xt[ci][:].rearrange("p (hq c) -> p hq c", c=C),
                    axis=mybir.AxisListType.X,
                ).then_inc(dve_sem, 1)
                vector.wait_ge(dve_sem, b + 1)
                vector.reduce_max(
                    out=delta[ci][:],
                    in_=red2[:, : H * tt].rearrange("p (h q) -> p q h", h=H),
                    axis=mybir.AxisListType.X,
                ).then_inc(dve_sem, 1)
                vector.wait_ge(dve_sem, b + 2)
                vector.reciprocal(inv[ci][:], delta[ci][:]).then_inc(dve_sem, 1)
                vector.wait_ge(dve_sem, b + 3)
                # inv' = isqrt2/delta (M1 scale); d2 = 2*delta (M2 scale)
                vector.tensor_scalar_mul(inv[ci][:], inv[ci][:], ISQRT2).then_inc(
                    scal_sem, 1
                )
                vector.tensor_scalar_mul(d2[ci][:], delta[ci][:], 2.0).then_inc(
                    scal_sem, 1
                )
                b += 3
                if ci >= 1:
                    and_m2a(ci - 1)
            # (a last-chunk all-DVE fast path measured WORSE: +4 slices on
            # the co-critical DVE cost more than the zigzag latency saved)
            and_m2a(n_chunks - 1)

        @block.scalar
        def _(scalar):
            def m2b(k):
                # M2 slices m2d..tt-1: out = p2 * d2, xt -> wt
                for s in range(m2d[k], tts[k]):
                    scalar.activation(
                        out=view4(wt[k], k)[:, :, s, :],
                        in_=view4(xt[k], k)[:, :, s, :],
                        func=AF.Copy,
                        scale=d2[k][:, s : s + 1],
                    ).then_inc(m2b_sem, 1)

            def store(k):
                # BOTH fences are data fences: in-stream order does NOT
                # imply the DMA reads completed data (engines pipeline).
                # NOTE gpsimd is NOT usable for stores: its queue is
                # software-DGE (qGpSimdDynamic, no -HW suffix) -- slow
                # descriptor writes and corrupted output when tried.
                scalar.wait_ge(m2a_sem, cum(m2d, k))
                scalar.wait_ge(m2b_sem, cum(cm2b, k))
                scalar.dma_start(out=dst_ap(k), in_=wt[k][:]).then_inc(
                    store_sem, 16
                )

            def m1(ci):
                # M1: q = x * inv' (xt -> wt)
                scalar.wait_ge(scal_sem, 2 * (ci + 1))
                for s in range(tts[ci]):
                    scalar.activation(
                        out=view4(wt[ci], ci)[:, :, s, :],
                        in_=view4(xt[ci], ci)[:, :, s, :],
                        func=AF.Copy,
                        scale=inv[ci][:, s : s + 1],
                    ).then_inc(m1_sem, 1)

            for ci in range(n_chunks):
                m1(ci)
                if ci >= 1:
                    scalar.wait_ge(and_sem, ci)
                    m2b(ci - 1)
                    store(ci - 1)
            scalar.wait_ge(and_sem, n_chunks)
            m2b(n_chunks - 1)
            store(n_chunks - 1)

    _nc_cache["nc"] = nc
    return nc


def kernel(x: np.ndarray) -> np.ndarray:
    assert x.shape == (B, H, T, C) and x.dtype == np.float32
    nc = _build_nc()
    in_maps = [{"x": np.ascontiguousarray(x[i])} for i in range(N_CORES)]
    res = run_bass_kernel_spmd(nc, in_maps, list(range(N_CORES)))
    out = np.stack([res.results[i]["y"] for i in range(N_CORES)], axis=0)
    return out


# revision 56
# speedup vs baseline: 1.0378x; 1.0378x over previous
"""Log2Quantizer Trainium2 kernel (raw Bass, no Tile).

Math: the reference's sort/std/rank machinery is dead code (bit_token is
unconditionally overwritten with n_bits), so the computation reduces to:
    delta[b,t] = max over (h,c) of x[b,h,t,c]
    out = delta * 2^(round(log2(max(x/delta, 1e-8))))
i.e. snap x/delta to the nearest power of two in log space, rescale by delta.

Bit-trick (no transcendentals): round(log2 r) = floor(log2(r/sqrt2)) + 1:
    q   = x * (isqrt2/delta)                 per-token scale (ACT, M1)
    p2  = bitcast_f32(bits(q) & 0x7F800000)  2^floor(log2 q)   (DVE, AND)
    out = p2 * (2*delta)                     exact fp32 mult   (M2, split)
x==0 gives q=0 -> p2=+0.0 -> out=0 (reference's clamp yields delta*2^-27
~ 7e-9 there; abs err 7e-9 on the rare exact-zero input).

Sharding: data-parallel over batch dim b (8 rows -> 8 cores), no comms.
Layout: t split into chunks; partition dim = t-block of tt=TC/128 so each
partition line is one contiguous tt*256B run per h in DRAM (>=512B keeps
DMA descriptors at the full 22.5 B/ns per-engine bus rate; 16 engines ~
360 GB/s aggregate is the hard bottleneck at 25.2 MB total traffic ->
~70us floor). Chunk sizes are VARIABLE: small 256-token chunks at the
start (first store enters the DMA mix ~20us earlier, so the load/store
streams share the engines for more of the run) and at the end (the tail
drains a small last store instead of a 1.6MB one).

Engine pipeline (vs the all-DVE baseline at 98.6us whose DVE ran 86us).
Per-token-scalar ops are sliced into tt per-q ops whose scalar is a
[128,1] AP; measured slice costs: DVE ~613ns, ACT ~1017ns per 512-chunk
slice.
  Sync:   load DMAs (own HWDGE ring), paced to <=3 in flight: an overfull
          ring makes dma_start block and throttles the transfers (measured
          260 GB/s un-paced vs 350 paced). One SBUF buffer pair per chunk
          (whole tensor resident), so loads wait on nothing else.
  DVE:    2-stage max-reduce (contiguous-X over c at 2 elem/cyc, then the
          tiny strided reduce over h), reciprocal, 2 tiny per-token ops,
          flat single-op AND, M2 slices 0..m2d-1.
  ACT:    M1 (all tt slices) + M2 slices m2d..tt-1 (activation-Copy,
          per-partition scale AP) + store DMA issue (own HWDGE ring).
Per 512-chunk: DVE ~7.5us, ACT ~7.1us, within the ~8.1us/chunk DMA share,
so in steady state the DMA rings are the pacer, not compute.

Buffers ping-pong, no in-place ops, one pair per chunk so no reuse:
M1 xt->wt, AND wt->xt, M2 xt->wt, store from wt.

Sems -- every data handoff (same-engine included: engines pipeline, e.g.
an issued DMA can read an earlier op's output before its data lands)
waits on the producer's counting increment (cumulative over chunks since
slice counts vary):
  load_sem:  +16 per load DMA; DVE waits 16*(ci+1) (one FIFO ring, in-order)
  scal_sem:  +2 per chunk by DVE (inv', d2); ACT M1 waits 2*(ci+1)
  m1_sem:    +1 per ACT M1 slice; DVE AND waits cum_m1(ci)
  and_sem:   +1 per chunk by DVE AND; ACT M2b waits ci+1
  m2a_sem:   +1 per DVE M2 slice;  ACT store waits cum_m2a(ci)
  m2b_sem:   +1 per ACT M2 slice;  ACT store self-waits cum_m2b(ci)
  dve_sem:   DVE-internal RAW fences (reduce1->reduce2->recip->tinies)
  store_sem: +16 per store DMA; sync tail-waits 16*n_chunks (output flushed)
"""

from contextlib import ExitStack

import numpy as np

import concourse.bass as bass
import concourse.mybir as mybir
from concourse.bass_utils import run_bass_kernel_spmd

B, H, T, C = 8, 12, 4096, 64
N_CORES = 8
P = 128          # SBUF partitions

# chunk sizes in tokens; sum must be T. Small chunks at both ends: the
# first store enters the DMA mix early, and the tail drains a small store.
# minimum 256 tokens: tt=2 keeps per-(partition,h) DRAM runs at 512B, the
# smallest size that still runs the DMA descriptors at full bus rate
CHUNKS = [256, 256, 512, 512, 512, 512, 512, 512, 256, 256]
assert sum(CHUNKS) == T

ISQRT2 = 0.7071067811865476
EXP_MASK = 0x7F800000

_nc_cache = {}


def _build_nc():
    if "nc" in _nc_cache:
        return _nc_cache["nc"]
    f32 = mybir.dt.float32
    i32 = mybir.dt.int32
    OP = mybir.AluOpType
    AF = mybir.ActivationFunctionType

    nc = bass.Bass()
    x_in = nc.declare_dram_parameter("x", [H, T, C], f32, isOutput=False)
    y_out = nc.declare_dram_parameter("y", [H, T, C], f32, isOutput=True)

    n_chunks = len(CHUNKS)
    offs = [sum(CHUNKS[:i]) for i in range(n_chunks)]
    tts = [tc // P for tc in CHUNKS]
    m2d = [tt // 2 for tt in tts]          # M2 slices on DVE per chunk

    def cum(xs, i):
        # total of xs[0..i] inclusive
        return sum(xs[: i + 1])

    def src_ap(ci):
        return x_in[:, offs[ci] : offs[ci] + CHUNKS[ci], :].rearrange(
            "h (p q) c -> p h (q c)", p=P
        )

    def dst_ap(ci):
        return y_out[:, offs[ci] : offs[ci] + CHUNKS[ci], :].rearrange(
            "h (p q) c -> p h (q c)", p=P
        )

    with ExitStack() as ctx:
        xt = [
            ctx.enter_context(
                nc.sbuf_tensor(f"xt{j}", [P, H * tts[j] * C], f32)
            )
            for j in range(n_chunks)
        ]
        wt = [
            ctx.enter_context(
                nc.sbuf_tensor(f"wt{j}", [P, H * tts[j] * C], f32)
            )
            for j in range(n_chunks)
        ]
        # scratch for the c-reduce stage
        red2 = ctx.enter_context(
            nc.sbuf_tensor("red2", [P, H * max(tts)], f32)
        )
        delta = [
            ctx.enter_context(nc.sbuf_tensor(f"delta{j}", [P, tts[j]], f32))
            for j in range(n_chunks)
        ]
        inv = [
            ctx.enter_context(nc.sbuf_tensor(f"inv{j}", [P, tts[j]], f32))
            for j in range(n_chunks)
        ]
        d2 = [
            ctx.enter_context(nc.sbuf_tensor(f"d2_{j}", [P, tts[j]], f32))
            for j in range(n_chunks)
        ]

        load_sem = ctx.enter_context(nc.semaphore("load_sem"))
        store_sem = ctx.enter_context(nc.semaphore("store_sem"))
        scal_sem = ctx.enter_context(nc.semaphore("scal_sem"))
        m1_sem = ctx.enter_context(nc.semaphore("m1_sem"))
        and_sem = ctx.enter_context(nc.semaphore("and_sem"))
        m2a_sem = ctx.enter_context(nc.semaphore("m2a_sem"))
        m2b_sem = ctx.enter_context(nc.semaphore("m2b_sem"))
        dve_sem = ctx.enter_context(nc.semaphore("dve_sem"))

        block = ctx.enter_context(nc.Block())

        def view4(t, ci):
            return t[:].rearrange("p (h q c) -> p h q c", h=H, c=C)

        cm2b = [tts[i] - m2d[i] for i in range(n_chunks)]

        @block.sync
        def _(sync):
            # loads only; SP HWDGE ring (two-ring DMA splits tested net-
            # negative: the ~350 GB/s single-stream cap is an HBM R/W-
            # direction effect, not per-ring -- mixed R+W reaches ~390)
            for ci in range(n_chunks):
                if ci >= 4:
                    sync.wait_ge(load_sem, 16 * (ci - 3))
                sync.dma_start(out=xt[ci][:], in_=src_ap(ci)).then_inc(
                    load_sem, 16
                )
            # output-flush guarantee before NEFF end
            sync.wait_ge(store_sem, 16 * n_chunks)

        @block.vector
        def _(vector):
            def and_m2a(k):
                # AND: p2 = bits(q) & mask, wt -> xt, one flat op
                vector.wait_ge(m1_sem, cum(tts, k))
                vector.tensor_scalar(
                    out=xt[k][:].bitcast(i32),
                    in0=wt[k][:].bitcast(i32),
                    scalar1=EXP_MASK,
                    scalar2=None,
                    op0=OP.bitwise_and,
                ).then_inc(and_sem, 1)
                # M2 slices 0..m2d-1: out = p2 * d2, xt -> wt
                vector.wait_ge(and_sem, k + 1)
                for s in range(m2d[k]):
                    vector.tensor_scalar_mul(
                        view4(wt[k], k)[:, :, s, :],
                        view4(xt[k], k)[:, :, s, :],
                        d2[k][:, s : s + 1],
                    ).then_inc(m2a_sem, 1)

            b = 0
            for ci in range(n_chunks):
                tt = tts[ci]
                vector.wait_ge(load_sem, 16 * (ci + 1))
                # delta = max over (h, c) in two stages: contiguous X over
                # c, then the tiny strided reduce over h. (Alternatives
                # measured SLOWER: transposed XY reduce 3346ns/chunk,
                # tensor_scalar+accum 'CACHE_REDUCE' 948ns/slice, h-fold
                # via tensor_max 1432ns + half-reduce; this is 3584ns.)
                vector.reduce_max(
                    out=red2[:, : H * tt],
                    in_=xt[ci][:].rearrange("p (hq c) -> p hq c", c=C),
                    axis=mybir.AxisListType.X,
                ).then_inc(dve_sem, 1)
                vector.wait_ge(dve_sem, b + 1)
                vector.reduce_max(
                    out=delta[ci][:],
                    in_=red2[:, : H * tt].rearrange("p (h q) -> p q h", h=H),
                    axis=mybir.AxisListType.X,
                ).then_inc(dve_sem, 1)
                vector.wait_ge(dve_sem, b + 2)
                vector.reciprocal(inv[ci][:], delta[ci][:]).then_inc(dve_sem, 1)
                vector.wait_ge(dve_sem, b + 3)
                # inv' = isqrt2/delta (M1 scale); d2 = 2*delta (M2 scale)
                vector.tensor_scalar_mul(inv[ci][:], inv[ci][:], ISQRT2).then_inc(
                    scal_sem, 1
                )
                vector.tensor_scalar_mul(d2[ci][:], delta[ci][:], 2.0).then_inc(
                    scal_sem, 1
                )
                b += 3
                if ci >= 1:
                    and_m2a(ci - 1)
            # (a last-chunk all-DVE fast path measured WORSE: +4 slices on
            # the co-critical DVE cost more than the zigzag latency saved)
            and_m2a(n_chunks - 1)

        @block.scalar
        def _(scalar):
            def m2b(k):
                # M2 slices m2d..tt-1: out = p2 * d2, xt -> wt
                for s in range(m2d[k], tts[k]):
                    scalar.activation(
                        out=view4(wt[k], k)[:, :, s, :],
                        in_=view4(xt[k], k)[:, :, s, :],
                        func=AF.Copy,
                        scale=d2[k][:, s : s + 1],
                    ).then_inc(m2b_sem, 1)

            def store(k):
                # BOTH fences are data fences: in-stream order does NOT
                # imply the DMA reads completed data (engines pipeline).
                # NOTE gpsimd is NOT usable for stores: its queue is
                # software-DGE (qGpSimdDynamic, no -HW suffix) -- slow
                # descriptor writes and corrupted output when tried.
                scalar.wait_ge(m2a_sem, cum(m2d, k))
                scalar.wait_ge(m2b_sem, cum(cm2b, k))
                scalar.dma_start(out=dst_ap(k), in_=wt[k][:]).then_inc(
                    store_sem, 16
                )

            def m1(ci):
                # M1: q = x * inv' (xt -> wt)
                scalar.wait_ge(scal_sem, 2 * (ci + 1))
                for s in range(tts[ci]):
                    scalar.activation(
                        out=view4(wt[ci], ci)[:, :, s, :],
                        in_=view4(xt[ci], ci)[:, :, s, :],
                        func=AF.Copy,
                        scale=inv[ci][:, s : s + 1],
                    ).then_inc(m1_sem, 1)

            for ci in range(n_chunks):
                m1(ci)
                if ci >= 1:
                    scalar.wait_ge(and_sem, ci)
                    m2b(ci - 1)
                    store(ci - 1)
            scalar.wait_ge(and_sem, n_chunks)
            m2b(n_chunks - 1)
            store(n_chunks - 1)

    _nc_cache["nc"] = nc
    return nc


def kernel(x: np.ndarray) -> np.ndarray:
    assert x.shape == (B, H, T, C) and x.dtype == np.float32
    nc = _build_nc()
    in_maps = [{"x": np.ascontiguousarray(x[i])} for i in range(N_CORES)]
    res = run_bass_kernel_spmd(nc, in_maps, list(range(N_CORES)))
    out = np.stack([res.results[i]["y"] for i in range(N_CORES)], axis=0)
    return out


# revision 57
# speedup vs baseline: 1.0423x; 1.0043x over previous
"""Log2Quantizer Trainium2 kernel (raw Bass, no Tile).

Math: the reference's sort/std/rank machinery is dead code (bit_token is
unconditionally overwritten with n_bits), so the computation reduces to:
    delta[b,t] = max over (h,c) of x[b,h,t,c]
    out = delta * 2^(round(log2(max(x/delta, 1e-8))))
i.e. snap x/delta to the nearest power of two in log space, rescale by delta.

Bit-trick (no transcendentals): round(log2 r) = floor(log2(r/sqrt2)) + 1:
    q   = x * (isqrt2/delta)                 per-token scale (ACT, M1)
    p2  = bitcast_f32(bits(q) & 0x7F800000)  2^floor(log2 q)   (DVE, AND)
    out = p2 * (2*delta)                     exact fp32 mult   (M2, split)
x==0 gives q=0 -> p2=+0.0 -> out=0 (reference's clamp yields delta*2^-27
~ 7e-9 there; abs err 7e-9 on the rare exact-zero input).

Sharding: data-parallel over batch dim b (8 rows -> 8 cores), no comms.
Layout: t split into chunks; partition dim = t-block of tt=TC/128 so each
partition line is one contiguous tt*256B run per h in DRAM (>=512B keeps
DMA descriptors at the full 22.5 B/ns per-engine bus rate; 16 engines ~
360 GB/s aggregate is the hard bottleneck at 25.2 MB total traffic ->
~70us floor). Chunk sizes are VARIABLE: small 256-token chunks at the
start (first store enters the DMA mix ~20us earlier, so the load/store
streams share the engines for more of the run) and at the end (the tail
drains a small last store instead of a 1.6MB one).

Engine pipeline (vs the all-DVE baseline at 98.6us whose DVE ran 86us).
Per-token-scalar ops are sliced into tt per-q ops whose scalar is a
[128,1] AP; measured slice costs: DVE ~613ns, ACT ~1017ns per 512-chunk
slice.
  Sync:   load DMAs (own HWDGE ring), paced to <=4 in flight: an overfull
          ring makes dma_start block and throttles the transfers (measured
          260 GB/s un-paced vs 350 paced). One SBUF buffer pair per chunk
          (whole tensor resident), so loads wait on nothing else.
  DVE:    2-stage max-reduce (contiguous-X over c at 2 elem/cyc, then the
          tiny strided reduce over h), reciprocal, 2 tiny per-token ops,
          flat single-op AND, M2 slices 0..m2d-1.
  ACT:    M1 (all tt slices) + M2 slices m2d..tt-1 (activation-Copy,
          per-partition scale AP) + store DMA issue (own HWDGE ring).
Per 512-chunk: DVE ~7.5us, ACT ~7.1us, within the ~8.1us/chunk DMA share,
so in steady state the DMA rings are the pacer, not compute.

Buffers ping-pong, no in-place ops, one pair per chunk so no reuse:
M1 xt->wt, AND wt->xt, M2 xt->wt, store from wt.

Sems -- every data handoff (same-engine included: engines pipeline, e.g.
an issued DMA can read an earlier op's output before its data lands)
waits on the producer's counting increment (cumulative over chunks since
slice counts vary):
  load_sem:  +16 per load DMA; DVE waits 16*(ci+1) (one FIFO ring, in-order)
  scal_sem:  +2 per chunk by DVE (inv', d2); ACT M1 waits 2*(ci+1)
  m1_sem:    +1 per ACT M1 slice; DVE AND waits cum_m1(ci)
  and_sem:   +1 per chunk by DVE AND; ACT M2b waits ci+1
  m2a_sem:   +1 per DVE M2 slice;  ACT store waits cum_m2a(ci)
  m2b_sem:   +1 per ACT M2 slice;  ACT store self-waits cum_m2b(ci)
  dve_sem:   DVE-internal RAW fences (reduce1->reduce2->recip->tinies)
  store_sem: +16 per store DMA; sync tail-waits 16*n_chunks (output flushed)
"""

from contextlib import ExitStack

import numpy as np

import concourse.bass as bass
import concourse.mybir as mybir
from concourse.bass_utils import run_bass_kernel_spmd

B, H, T, C = 8, 12, 4096, 64
N_CORES = 8
P = 128          # SBUF partitions

# chunk sizes in tokens; sum must be T. Small chunks at both ends: the
# first store enters the DMA mix early, and the tail drains a small store.
# minimum 256 tokens: tt=2 keeps per-(partition,h) DRAM runs at 512B, the
# smallest size that still runs the DMA descriptors at full bus rate
CHUNKS = [256, 256, 512, 512, 512, 512, 512, 512, 256, 256]
assert sum(CHUNKS) == T

ISQRT2 = 0.7071067811865476
EXP_MASK = 0x7F800000

_nc_cache = {}


def _build_nc():
    if "nc" in _nc_cache:
        return _nc_cache["nc"]
    f32 = mybir.dt.float32
    i32 = mybir.dt.int32
    OP = mybir.AluOpType
    AF = mybir.ActivationFunctionType

    nc = bass.Bass()
    x_in = nc.declare_dram_parameter("x", [H, T, C], f32, isOutput=False)
    y_out = nc.declare_dram_parameter("y", [H, T, C], f32, isOutput=True)

    n_chunks = len(CHUNKS)
    offs = [sum(CHUNKS[:i]) for i in range(n_chunks)]
    tts = [tc // P for tc in CHUNKS]
    m2d = [tt // 2 for tt in tts]          # M2 slices on DVE per chunk

    def cum(xs, i):
        # total of xs[0..i] inclusive
        return sum(xs[: i + 1])

    def src_ap(ci):
        return x_in[:, offs[ci] : offs[ci] + CHUNKS[ci], :].rearrange(
            "h (p q) c -> p h (q c)", p=P
        )

    def dst_ap(ci):
        return y_out[:, offs[ci] : offs[ci] + CHUNKS[ci], :].rearrange(
            "h (p q) c -> p h (q c)", p=P
        )

    with ExitStack() as ctx:
        xt = [
            ctx.enter_context(
                nc.sbuf_tensor(f"xt{j}", [P, H * tts[j] * C], f32)
            )
            for j in range(n_chunks)
        ]
        wt = [
            ctx.enter_context(
                nc.sbuf_tensor(f"wt{j}", [P, H * tts[j] * C], f32)
            )
            for j in range(n_chunks)
        ]
        # scratch for the c-reduce stage
        red2 = ctx.enter_context(
            nc.sbuf_tensor("red2", [P, H * max(tts)], f32)
        )
        delta = [
            ctx.enter_context(nc.sbuf_tensor(f"delta{j}", [P, tts[j]], f32))
            for j in range(n_chunks)
        ]
        inv = [
            ctx.enter_context(nc.sbuf_tensor(f"inv{j}", [P, tts[j]], f32))
            for j in range(n_chunks)
        ]
        d2 = [
            ctx.enter_context(nc.sbuf_tensor(f"d2_{j}", [P, tts[j]], f32))
            for j in range(n_chunks)
        ]

        load_sem = ctx.enter_context(nc.semaphore("load_sem"))
        store_sem = ctx.enter_context(nc.semaphore("store_sem"))
        scal_sem = ctx.enter_context(nc.semaphore("scal_sem"))
        m1_sem = ctx.enter_context(nc.semaphore("m1_sem"))
        and_sem = ctx.enter_context(nc.semaphore("and_sem"))
        m2a_sem = ctx.enter_context(nc.semaphore("m2a_sem"))
        m2b_sem = ctx.enter_context(nc.semaphore("m2b_sem"))
        dve_sem = ctx.enter_context(nc.semaphore("dve_sem"))

        block = ctx.enter_context(nc.Block())

        def view4(t, ci):
            return t[:].rearrange("p (h q c) -> p h q c", h=H, c=C)

        cm2b = [tts[i] - m2d[i] for i in range(n_chunks)]

        @block.sync
        def _(sync):
            # loads only; SP HWDGE ring (two-ring DMA splits tested net-
            # negative: the ~350 GB/s single-stream cap is an HBM R/W-
            # direction effect, not per-ring -- mixed R+W reaches ~390)
            for ci in range(n_chunks):
                if ci >= 4:
                    sync.wait_ge(load_sem, 16 * (ci - 3))
                sync.dma_start(out=xt[ci][:], in_=src_ap(ci)).then_inc(
                    load_sem, 16
                )
            # output-flush guarantee before NEFF end
            sync.wait_ge(store_sem, 16 * n_chunks)

        @block.vector
        def _(vector):
            def and_m2a(k):
                # AND: p2 = bits(q) & mask, wt -> xt, one flat op
                vector.wait_ge(m1_sem, cum(tts, k))
                vector.tensor_scalar(
                    out=xt[k][:].bitcast(i32),
                    in0=wt[k][:].bitcast(i32),
                    scalar1=EXP_MASK,
                    scalar2=None,
                    op0=OP.bitwise_and,
                ).then_inc(and_sem, 1)
                # M2 slices 0..m2d-1: out = p2 * d2, xt -> wt
                vector.wait_ge(and_sem, k + 1)
                for s in range(m2d[k]):
                    vector.tensor_scalar_mul(
                        view4(wt[k], k)[:, :, s, :],
                        view4(xt[k], k)[:, :, s, :],
                        d2[k][:, s : s + 1],
                    ).then_inc(m2a_sem, 1)

            b = 0
            for ci in range(n_chunks):
                tt = tts[ci]
                vector.wait_ge(load_sem, 16 * (ci + 1))
                # delta = max over (h, c) in two stages: contiguous X over
                # c, then the tiny strided reduce over h. (Alternatives
                # measured SLOWER: transposed XY reduce 3346ns/chunk,
                # tensor_scalar+accum 'CACHE_REDUCE' 948ns/slice, h-fold
                # via tensor_max 1432ns + half-reduce; this is 3584ns.)
                vector.reduce_max(
                    out=red2[:, : H * tt],
                    in_=xt[ci][:].rearrange("p (hq c) -> p hq c", c=C),
                    axis=mybir.AxisListType.X,
                ).then_inc(dve_sem, 1)
                vector.wait_ge(dve_sem, b + 1)
                vector.reduce_max(
                    out=delta[ci][:],
                    in_=red2[:, : H * tt].rearrange("p (h q) -> p q h", h=H),
                    axis=mybir.AxisListType.X,
                ).then_inc(dve_sem, 1)
                vector.wait_ge(dve_sem, b + 2)
                vector.reciprocal(inv[ci][:], delta[ci][:]).then_inc(dve_sem, 1)
                vector.wait_ge(dve_sem, b + 3)
                # inv' = isqrt2/delta (M1 scale); d2 = 2*delta (M2 scale)
                vector.tensor_scalar_mul(inv[ci][:], inv[ci][:], ISQRT2).then_inc(
                    scal_sem, 1
                )
                vector.tensor_scalar_mul(d2[ci][:], delta[ci][:], 2.0).then_inc(
                    scal_sem, 1
                )
                b += 3
                if ci >= 1:
                    and_m2a(ci - 1)
            # (a last-chunk all-DVE fast path measured WORSE: +4 slices on
            # the co-critical DVE cost more than the zigzag latency saved)
            and_m2a(n_chunks - 1)

        @block.scalar
        def _(scalar):
            def m2b(k):
                # M2 slices m2d..tt-1: out = p2 * d2, xt -> wt
                for s in range(m2d[k], tts[k]):
                    scalar.activation(
                        out=view4(wt[k], k)[:, :, s, :],
                        in_=view4(xt[k], k)[:, :, s, :],
                        func=AF.Copy,
                        scale=d2[k][:, s : s + 1],
                    ).then_inc(m2b_sem, 1)

            def store(k):
                # BOTH fences are data fences: in-stream order does NOT
                # imply the DMA reads completed data (engines pipeline).
                # NOTE gpsimd is NOT usable for stores: its queue is
                # software-DGE (qGpSimdDynamic, no -HW suffix) -- slow
                # descriptor writes and corrupted output when tried.
                scalar.wait_ge(m2a_sem, cum(m2d, k))
                scalar.wait_ge(m2b_sem, cum(cm2b, k))
                scalar.dma_start(out=dst_ap(k), in_=wt[k][:]).then_inc(
                    store_sem, 16
                )

            def m1(ci):
                # M1: q = x * inv' (xt -> wt)
                scalar.wait_ge(scal_sem, 2 * (ci + 1))
                for s in range(tts[ci]):
                    scalar.activation(
                        out=view4(wt[ci], ci)[:, :, s, :],
                        in_=view4(xt[ci], ci)[:, :, s, :],
                        func=AF.Copy,
                        scale=inv[ci][:, s : s + 1],
                    ).then_inc(m1_sem, 1)

            for ci in range(n_chunks):
                m1(ci)
                if ci >= 1:
                    scalar.wait_ge(and_sem, ci)
                    m2b(ci - 1)
                    store(ci - 1)
            scalar.wait_ge(and_sem, n_chunks)
            m2b(n_chunks - 1)
            store(n_chunks - 1)

    _nc_cache["nc"] = nc
    return nc


def kernel(x: np.ndarray) -> np.ndarray:
    assert x.shape == (B, H, T, C) and x.dtype == np.float32
    nc = _build_nc()
    in_maps = [{"x": np.ascontiguousarray(x[i])} for i in range(N_CORES)]
    res = run_bass_kernel_spmd(nc, in_maps, list(range(N_CORES)))
    out = np.stack([res.results[i]["y"] for i in range(N_CORES)], axis=0)
    return out
